# revision 20
# baseline (speedup 1.0000x reference)
"""Trainium2 Bass kernel for the lifted-structure metric loss (nn_Metric_Loss).

Math (reference): for X in {T (text), Z (interleaved text/shape)}:
    D = X @ X.T;  E = exp(0.5 + D)
    pair p (rows 2p, 2p+1): S[p] = sum(E[{i,j}, :]) - sum(E[{i,j},{i,j}])
    J[p] = relu(log(S[p]) - D[i,j])^2;  loss = mean(J)/2; total = l_T + 2 l_Z

W-formulation: de-interleave U = T[0::2], V = T[1::2], S = shapes (each
[2048, 1024]).  Both losses decompose over gram blocks of W = [U; V; S]:
loss T needs E_UU, E_UV, E_VV row/col sums; loss Z needs E_UU, E_US, E_SS
— the UU blocks are SHARED.  Corrections and the positive-pair sims D_ij
are exact host-side dot products (fp32 inputs), so no masks on device.

Work = 62 gram 512-blocks (vertices U0-3,V0-3,S0-3; all pairs except
V-S).  Dealt 8 cores x 8 slots with ZERO padding: per core 6 full slots
(F=512), one fractional slot (F=384, a slice of a 3-block "fan"), and a
symmetric-diagonal slot (row sums only).  Uniform SPMD program: fixed
slot->buffer-index pattern; the host fills each core's 8 aliased operand
buffers + the packed fan slice.

Per slot: 16 fp8 DoubleRow matmuls (K=256 each) -> PSUM [128,F];
ACT exp(bias 0.5) -> bf16 esc tiles.  Row sums are hybrid: m-tiles 0-1
via one DVE free-dim reduce, m-tiles 2-3 via the ACT accumulator
(READ_ACCUMULATOR), balancing the scalar and vector engines under the
PE pace.  Col sums via DVE/GpSimd adds of the 4 exp tiles, shipped bf16
for host partition reduction.  Host does O(N) assembly in float64.

Schedule (the PE p-state ramps to full clock only after ~3us of
continuous activity, and each HWDGE ring sustains only ~0.2 GB/ms):
  - f32 dummy matmuls warm the PE clock while the first pieces stream;
  - slot-0 operands stream as k-chunk pieces on BOTH HWDGE rings (sync
    carries b0, scalar carries b2 + b3), slot 0 runs k-major so the
    first matmul needs only chunks 0-1;
  - the diagonal slot (needs no new data) is split: half runs SECOND,
    buying ~1.7us of streaming slack for b3, half runs LAST so the
    final dependency chain is matmul -> ACT -> READ_ACC -> tiny DMA.
"""

import numpy as np
import ml_dtypes

import concourse.mybir as mybir
import concourse.tile as tile
from concourse import bacc
from concourse.bass import ds
from concourse.bass_utils import run_bass_kernel_spmd

N, D_EMB = 4096, 1024
P_PAIRS = N // 2
NCORES = 8
B = 512                    # block size (vertex rows)
KC = D_EMB // 128          # 8 k-chunks
MT = B // 128              # 4 m-tiles per slot
NSLOTS = 8
FRAC_SLOT = 6
DIAG_SLOT = 7
F_FRAC = 384               # fractional slot free dim
MARGIN = 0.5
NBUF = 8
# PE clock warm-up: f32 dummy matmuls issued while slot-0 operands stream
WARM_NS = [128, 128, 64, 64, 64, 64, 64, 64, 64, 64]

# vertices 0-3 = U0-3, 4-7 = V0-3, 8-11 = S0-3
# slot -> (lhs buf index, rhs buf index); slot 6 rhs is the fan buffer FR
SLOT_PATTERN = [(0, 2), (0, 3), (1, 2), (1, 3), (0, 4), (5, 6), (7, None), (2, 2)]
# per-core vertex content of buffers b0..b7 (aliases allowed)
BUFV = [
    [0, 1, 4, 5, 0, 0, 3, 4],
    [2, 3, 6, 7, 2, 1, 3, 4],
    [1, 0, 7, 6, 1, 5, 6, 4],
    [3, 2, 5, 4, 3, 5, 7, 4],
    [0, 1, 8, 9, 1, 6, 7, 8],
    [2, 3, 10, 11, 3, 9, 10, 8],
    [0, 1, 11, 10, 2, 9, 11, 8],
    [2, 3, 9, 8, 1, 10, 11, 8],
]
# fans: cores 0-3 compute rows V0 x cols V1|V2|V3 slice [384c:384c+384];
# cores 4-7 rows S0 x cols S1|S2|S3 slice [384(c-4):...].
V_FAN = (5, 6, 7)
S_FAN = (9, 10, 11)

# execution phases: (slot, tiles).  The diag slot (only needs b2, which
# streams first) is split: tiles 0-1 interleave INTO slot 0's k-groups as
# filler while operand pieces stream; tiles 2-3 run LAST so the final
# dependency chain is matmul -> ACT -> READ_ACC -> tiny DMA with no
# col-sum adds or big output transfers behind it.
PHASES = (
    [(0, (0, 1, 2, 3))]
    + [(s, (0, 1, 2, 3)) for s in range(1, NSLOTS - 1)]
    + [(DIAG_SLOT, (2, 3))]
)


def _slot_edges(c):
    """[(row_vertex, col_vertex_or_None_for_frac)] for core c's 8 slots."""
    out = []
    for s, (l, r) in enumerate(SLOT_PATTERN):
        if s == FRAC_SLOT:
            out.append((BUFV[c][l], None))
        else:
            out.append((BUFV[c][l], BUFV[c][r]))
    return out


def _check_deal():
    need = set()
    for g in ((0, 1, 2, 3), (4, 5, 6, 7), (8, 9, 10, 11)):
        for i, a in enumerate(g):
            for b in g[i:]:
                need.add((a, b))
    for a in (0, 1, 2, 3):
        for b in range(4, 12):
            need.add((a, b))
    got = []
    for c in range(NCORES):
        for s, (a, b) in enumerate(_slot_edges(c)):
            if s == FRAC_SLOT:
                continue
            got.append((min(a, b), max(a, b)))
    fan_edges = [(5, 4), (6, 4), (7, 4), (9, 8), (10, 8), (11, 8)]
    got += [(min(a, b), max(a, b)) for a, b in fan_edges]
    assert sorted(got) == sorted(need), "deal does not cover the 62 blocks"


_check_deal()
_CACHE = {}


def _build_nc():
    nc = bacc.Bacc(
        "TRN2",
        target_bir_lowering=False,
        debug=False,
        num_devices=NCORES,
        enable_partition_id=False,
        monotonic_sem_count=0,
    )
    f32 = mybir.dt.float32
    bf16 = mybir.dt.bfloat16
    fp8 = mybir.dt.float8e4
    bufs_d = nc.dram_tensor("bufs", [NBUF, 128, KC, B], fp8, kind="ExternalInput").ap()
    fan_d = nc.dram_tensor("fan", [128, KC, F_FRAC], fp8, kind="ExternalInput").ap()
    # row sums: col s*4+t = m-tile t of slot s
    out_main = nc.dram_tensor(
        "out_main", [128, NSLOTS * MT], f32, kind="ExternalOutput"
    ).ap()
    # per-slot exp-tile sums (slots 0-6); host reduces partitions
    out_acc = nc.dram_tensor(
        "out_acc", [NSLOTS - 1, 128, B], bf16, kind="ExternalOutput"
    ).ap()
    # diag-slot triangle support: exp values above the tri tiles
    out_accd = nc.dram_tensor("out_accd", [128, 256], bf16, kind="ExternalOutput").ap()
    out_acct2 = nc.dram_tensor(
        "out_acct2", [128, 128], bf16, kind="ExternalOutput"
    ).ap()


    with tile.TileContext(nc) as tc:
        with (
            tc.tile_pool(name="xb", bufs=1) as xb_pool,
            tc.tile_pool(name="consts", bufs=1) as consts,
            tc.tile_pool(name="psum", bufs=2, space="PSUM") as psum_pool,
            tc.tile_pool(name="esc", bufs=3) as esc_pool,
            tc.tile_pool(name="stats", bufs=3) as stats,
        ):
            # constants first, on gpsimd (it runs the framework const
            # memsets first thing, so ours execute right behind them and
            # the PE warm-up dummies can start ~0.7us earlier)
            bias_sb = consts.tile([128, 1], f32, tag="bias")
            nc.gpsimd.memset(bias_sb, MARGIN)
            warm_g = consts.tile([128, 128], f32, tag="warm_g")
            nc.gpsimd.memset(warm_g, 0.0)
            rp_sb = consts.tile([128, NSLOTS * MT], f32, tag="rp")

            # operand buffers
            xbs = [
                xb_pool.tile([128, KC, B], fp8, name=f"xb{b}", tag=f"xb{b}")
                for b in range(NBUF)
            ]
            fan_sb = xb_pool.tile([128, KC, F_FRAC], fp8, tag="fan")

            # slot-0 operands stream as k-chunk pieces on BOTH HWDGE rings
            # (sync carries b0, scalar carries b2 then b3); the other
            # buffers stream whole, ordered by slot consumption time.
            # b5/b7 issue from the scalar ring later (between ACT groups).
            for h in range(KC // 2):
                nc.sync.dma_start(
                    out=xbs[0][:, 2 * h : 2 * h + 2, :],
                    in_=bufs_d[0, :, 2 * h : 2 * h + 2, :],
                )
                nc.scalar.dma_start(
                    out=xbs[2][:, 2 * h : 2 * h + 2, :],
                    in_=bufs_d[2, :, 2 * h : 2 * h + 2, :],
                )
            # b3 gates slot 1 right after the merged phase — split it
            # across both rings so it lands in half the time
            nc.sync.dma_start(out=xbs[3][:, 0:4, :], in_=bufs_d[3, :, 0:4, :])
            nc.scalar.dma_start(out=xbs[3][:, 4:8, :], in_=bufs_d[3, :, 4:8, :])
            for b in (1, 4, 6):
                nc.sync.dma_start(out=xbs[b], in_=bufs_d[b])
            nc.sync.dma_start(out=fan_sb, in_=fan_d)

            # warm the exp table set (forces the ACT_TABLE_LOAD early)
            warm = consts.tile([128, 1], f32, tag="warm")
            nc.scalar.activation(warm, bias_sb, mybir.ActivationFunctionType.Exp)

            # PE clock warm-up: f32 dummy matmuls on const data keep the PE
            # continuously busy from body start so the p-state ramp (full
            # clock after ~3us of activity) completes before real operands
            # arrive.  They write a scratch generation of the dps0 psum tag;
            # real slot matmuls start=True overwrite it.
            warm_ps = psum_pool.tile([128, B], f32, name="warm_ps", tag="dps0")
            for wn in WARM_NS:
                nc.tensor.matmul(
                    warm_ps[0:1, 0:wn],
                    bias_sb[:, 0:1],
                    warm_g[:, 0:wn],
                )

            def emit_act(slot, esc, lane, t, dpsum, F):
                """exp ACT for m-tile t; tiles >=2 row-sum via accumulator."""
                col = slot * MT + t
                if t >= 2:
                    nc.scalar.activation(
                        esc[:, lane, :F],
                        dpsum[:, :F],
                        mybir.ActivationFunctionType.Exp,
                        bias=bias_sb,
                        scale=1.0,
                        accum_out=rp_sb[:, col : col + 1],
                    )
                else:
                    nc.scalar.activation(
                        esc[:, lane, :F],
                        dpsum[:, :F],
                        mybir.ActivationFunctionType.Exp,
                        bias=bias_sb,
                        scale=1.0,
                    )

            # --- phase 0: slot 0 k-major, with diag tiles 0-1 (only need
            # b2, the first-streamed buffer) interleaved into the k-group
            # gaps so the PE never idles while pieces stream in.
            esc0 = esc_pool.tile([128, MT, B], bf16, tag="esc")
            escd = esc_pool.tile([128, MT, B], bf16, tag="esc")
            dps0 = {
                t: psum_pool.tile([128, B], f32, name=f"dps{t}", tag=f"dps{t}")
                for t in range(MT)
            }
            dpsd = {
                t: psum_pool.tile([128, B], f32, name=f"dpsd{t}", tag=f"dps{t}")
                for t in (0, 1)
            }
            xl0 = xbs[SLOT_PATTERN[0][0]]
            xr0 = xbs[SLOT_PATTERN[0][1]]
            xd = xbs[SLOT_PATTERN[DIAG_SLOT][0]]
            for k2 in range(KC // 2):
                last = k2 == KC // 2 - 1
                for t in range(MT):
                    nc.tensor.matmul(
                        dps0[t],
                        xl0[:, 2 * k2 : 2 * k2 + 2, ds(128 * t, 128)],
                        xr0[:, 2 * k2 : 2 * k2 + 2, :],
                        start=(k2 == 0),
                        stop=last,
                        perf_mode=mybir.MatmulPerfMode.DoubleRow,
                    )
                    if last:
                        emit_act(0, esc0, t, t, dps0[t], B)
                for t in (0, 1):
                    nc.tensor.matmul(
                        dpsd[t],
                        xd[:, 2 * k2 : 2 * k2 + 2, ds(128 * t, 128)],
                        xd[:, 2 * k2 : 2 * k2 + 2, :],
                        start=(k2 == 0),
                        stop=last,
                        perf_mode=mybir.MatmulPerfMode.DoubleRow,
                    )
                    if last:
                        emit_act(DIAG_SLOT, escd, t, t, dpsd[t], B)
            nc.vector.tensor_reduce(
                rp_sb[:, 0:2],
                esc0[:, 0:2, :],
                axis=mybir.AxisListType.X,
                op=mybir.AluOpType.add,
            )
            nc.vector.tensor_reduce(
                rp_sb[:, DIAG_SLOT * MT : DIAG_SLOT * MT + 2],
                escd[:, 0:2, :],
                axis=mybir.AxisListType.X,
                op=mybir.AluOpType.add,
            )
            # right half of the diag rows 0:256, summed — feeds the host's
            # below-diagonal column sums for the triangular tiles 2-3
            accd = stats.tile([128, 256], bf16, tag="accd")
            nc.vector.tensor_add(
                accd, escd[:, 0, 256:512], escd[:, 1, 256:512]
            )
            nc.sync.dma_start(out=out_accd, in_=accd)
            acc01 = stats.tile([128, B], bf16, tag="acc01")
            nc.vector.tensor_add(acc01, esc0[:, 0, :], esc0[:, 1, :])
            acc23 = stats.tile([128, B], bf16, tag="acc23")
            nc.gpsimd.tensor_add(acc23, esc0[:, 2, :], esc0[:, 3, :])
            accf = stats.tile([128, B], bf16, tag="accf")
            nc.vector.tensor_add(accf, acc01, acc23)
            nc.sync.dma_start(out=out_acc[0], in_=accf)

            # --- remaining phases: diag tiles 2-3 (covers the b3 stream),
            # then slots 1..6 m-tile-major; frac slot last.
            for p, (s, tiles) in enumerate(PHASES[1:]):
                li, ri = SLOT_PATTERN[s]
                xl = xbs[li]
                xr = fan_sb if s == FRAC_SLOT else xbs[ri]
                F = F_FRAC if s == FRAC_SLOT else B
                # late operand issues ride the scalar ring between ACT groups
                if p == 2:
                    nc.scalar.dma_start(out=xbs[5], in_=bufs_d[5])
                if p == 3:
                    nc.scalar.dma_start(out=xbs[7], in_=bufs_d[7])
                esc = esc_pool.tile([128, MT, B], bf16, tag="esc")
                dpsums = {}
                for t in tiles:
                    dpsums[t] = psum_pool.tile(
                        [128, B], f32, name=f"dps{t}", tag=f"dps{t}"
                    )
                for t in tiles:
                    if s == DIAG_SLOT:
                        # triangular tiles: only cols >= the tile's row
                        # block; the below-diagonal part comes from the
                        # shipped exp tiles by symmetry (host side)
                        c0, w = 128 * t, B - 128 * t
                    else:
                        c0, w = 0, F
                    for k2 in range(KC // 2):
                        nc.tensor.matmul(
                            dpsums[t][:, :w],
                            xl[:, 2 * k2 : 2 * k2 + 2, ds(128 * t, 128)],
                            xr[:, 2 * k2 : 2 * k2 + 2, ds(c0, w)],
                            start=(k2 == 0),
                            stop=(k2 == KC // 2 - 1),
                            perf_mode=mybir.MatmulPerfMode.DoubleRow,
                        )
                    emit_act(s, esc, t - tiles[0], t, dpsums[t], w)
                    if s == DIAG_SLOT and t == 2:
                        # rows 256:384 x cols 384:512 -> host col sums for
                        # the last tile's rows
                        nc.sync.dma_start(
                            out=out_acct2, in_=esc[:, 0, 128:256]
                        )
                # row sums for m-tiles 0-1 via one DVE free-dim reduce
                if tiles[0] == 0:
                    nc.vector.tensor_reduce(
                        rp_sb[:, s * MT : s * MT + 2],
                        esc[:, 0:2, :F],
                        axis=mybir.AxisListType.X,
                        op=mybir.AluOpType.add,
                    )
                # col sums (not needed for the diag slot)
                if s == DIAG_SLOT:
                    continue
                acc01 = stats.tile([128, B], bf16, tag="acc01")
                nc.vector.tensor_add(acc01[:, :F], esc[:, 0, :F], esc[:, 1, :F])
                acc23 = stats.tile([128, B], bf16, tag="acc23")
                # the frac slot is 2nd-to-last: its add chain gates the
                # final barrier, so keep it off the slow gpsimd
                eng23 = nc.vector if s == FRAC_SLOT else nc.gpsimd
                eng23.tensor_add(acc23[:, :F], esc[:, 2, :F], esc[:, 3, :F])
                accf = stats.tile([128, B], bf16, tag="accf")
                nc.vector.tensor_add(accf[:, :F], acc01[:, :F], acc23[:, :F])
                nc.sync.dma_start(out=out_acc[s][:, :F], in_=accf[:, :F])
            # row sums out: single DMA once the last READ_ACC lands; issued
            # from the scalar ring, which is idle right after that READ
            nc.scalar.dma_start(out=out_main, in_=rp_sb)
    nc.compile()
    return nc


def _get_nc():
    if "nc" not in _CACHE:
        _CACHE["nc"] = _build_nc()
    return _CACHE["nc"]


def _prep_host(text_embeddings, shape_embeddings):
    """-> (W fp64 [12*512, 1024], per-core in_maps)."""
    T = np.asarray(text_embeddings, dtype=np.float32)
    S = np.asarray(shape_embeddings, dtype=np.float32)
    W = np.concatenate([T[0::2], T[1::2], S], axis=0)  # [6144, 1024]
    # [128, KC, 6144] fp8 view of W^T: partition p, chunk kc = d-row kc*128+p
    WT = np.ascontiguousarray(W.T).astype(ml_dtypes.float8_e4m3)
    Wg = WT.reshape(KC, 128, 12 * B).transpose(1, 0, 2)
    Wg = np.ascontiguousarray(Wg)
    in_maps = []
    for c in range(NCORES):
        bufs = np.empty((NBUF, 128, KC, B), dtype=ml_dtypes.float8_e4m3)
        for b in range(NBUF):
            v = BUFV[c][b]
            bufs[b] = Wg[:, :, B * v : B * (v + 1)]
        fan_vs = V_FAN if c < 4 else S_FAN
        fan_cols = np.concatenate(
            [Wg[:, :, B * v : B * (v + 1)] for v in fan_vs], axis=2
        )
        k = (c % 4) * F_FRAC
        fan = np.ascontiguousarray(fan_cols[:, :, k : k + F_FRAC])
        in_maps.append({"bufs": bufs, "fan": fan})
    return W.astype(np.float64), in_maps


def _finalize(W, outs):
    """W: [6144,1024] fp64; outs: 8 per-core output dicts -> scalar loss."""
    # per-vertex row-sum accumulators for each loss
    rs = [np.zeros((12, B), np.float64), np.zeros((12, B), np.float64)]

    def classes(a, b):
        ga, gb = a // 4, b // 4
        if ga == 0 and gb == 0:
            return (0, 1)
        return (0,) if (ga <= 1 and gb <= 1) else (1,)

    for c, o in enumerate(outs):
        rp = np.asarray(o["out_main"], np.float64)          # [128, 32]
        acc = np.asarray(o["out_acc"], np.float32).astype(np.float64)
        cs = acc.sum(axis=1)                                 # [7, 512] col sums
        edges = _slot_edges(c)
        # diag-slot triangle: below-diagonal col sums from shipped exp tiles
        accd_s = np.asarray(o["out_accd"], np.float32).astype(np.float64).sum(axis=0)
        acct2_s = np.asarray(o["out_acct2"], np.float32).astype(np.float64).sum(axis=0)
        for s in range(NSLOTS):
            a, b = edges[s]
            F = F_FRAC if s == FRAC_SLOT else B
            # row sums -> vertex a
            for t in range(MT):
                seg = rp[:, s * MT + t]
                if s == DIAG_SLOT and t == 2:
                    seg = seg + accd_s[0:128]
                elif s == DIAG_SLOT and t == 3:
                    seg = seg + accd_s[128:256] + acct2_s
                if s == FRAC_SLOT:
                    for l in classes(a, a):
                        rs[l][a][128 * t : 128 * (t + 1)] += seg
                else:
                    for l in classes(a, b):
                        rs[l][a][128 * t : 128 * (t + 1)] += seg
            # col sums -> vertex b (skip diag blocks and the diag slot)
            if s == DIAG_SLOT:
                continue
            if s == FRAC_SLOT:
                fan_vs = V_FAN if c < 4 else S_FAN
                k = (c % 4) * F_FRAC
                for j0 in range(F):
                    g = k + j0
                    v = fan_vs[g // B]
                    for l in classes(a, v):
                        rs[l][v][g % B] += cs[s, j0]
            elif a != b:
                for l in classes(a, b):
                    rs[l][b] += cs[s, :F]

    U, V, S = W[0:2048], W[2048:4096], W[4096:6144]
    total = 0.0
    for l, (A_, B_) in enumerate(((U, V), (U, S))):
        dab = np.einsum("ij,ij->i", A_, B_)
        daa = np.einsum("ij,ij->i", A_, A_)
        dbb = np.einsum("ij,ij->i", B_, B_)
        rsB = rs[l][4:8].reshape(-1) if l == 0 else rs[l][8:12].reshape(-1)
        Sp = (
            rs[l][0:4].reshape(-1)
            + rsB
            - 2.0 * np.exp(MARGIN + dab)
            - np.exp(MARGIN + daa)
            - np.exp(MARGIN + dbb)
        )
        J = np.square(np.maximum(np.log(Sp) - dab, 0.0))
        loss = J.sum() / P_PAIRS / 2.0
        total += loss if l == 0 else 2.0 * loss
    return np.asarray(total, dtype=np.float32)


def kernel(text_embeddings, shape_embeddings):
    W, in_maps = _prep_host(text_embeddings, shape_embeddings)
    nc = _get_nc()
    res = run_bass_kernel_spmd(nc, in_maps, core_ids=list(range(NCORES)))
    return _finalize(W, res.results)


# revision 24
# speedup vs baseline: 1.1766x; 1.1766x over previous
"""Trainium2 Bass kernel for the lifted-structure metric loss (nn_Metric_Loss).

Math (reference): for X in {T (text), Z (interleaved text/shape)}:
    D = X @ X.T;  E = exp(0.5 + D)
    pair p (rows 2p, 2p+1): S[p] = sum(E[{i,j}, :]) - sum(E[{i,j},{i,j}])
    J[p] = relu(log(S[p]) - D[i,j])^2;  loss = mean(J)/2; total = l_T + 2 l_Z

W-formulation: de-interleave U = T[0::2], V = T[1::2], S = shapes (each
[2048, 1024]).  Both losses decompose over gram blocks of W = [U; V; S]:
loss T needs E_UU, E_UV, E_VV row/col sums; loss Z needs E_UU, E_US, E_SS
— the UU blocks are SHARED.  Corrections and the positive-pair sims D_ij
are exact host-side dot products (fp32 inputs), so no masks on device.

Work = 62 gram 512-blocks (vertices U0-3,V0-3,S0-3; all pairs except
V-S).  Dealt 8 cores x 8 slots with ZERO padding: per core 6 full slots
(F=512), one fractional slot (F=384, a slice of a 3-block "fan"), and a
symmetric-diagonal slot (row sums only).  Uniform SPMD program: fixed
slot->buffer-index pattern; the host fills each core's 8 aliased operand
buffers + the packed fan slice.

Per slot: 16 fp8 DoubleRow matmuls (K=256 each) -> PSUM [128,F];
ACT exp(bias 0.5) -> bf16 esc tiles.  Row sums are hybrid: m-tiles 0-1
via one DVE free-dim reduce, m-tiles 2-3 via the ACT accumulator
(READ_ACCUMULATOR), balancing the scalar and vector engines under the
PE pace.  Col sums via DVE/GpSimd adds of the 4 exp tiles, shipped bf16
for host partition reduction.  Host does O(N) assembly in float64.

Schedule (the PE p-state ramps to full clock only after ~3us of
continuous activity, and each HWDGE ring sustains only ~0.2 GB/ms):
  - f32 dummy matmuls warm the PE clock while the first pieces stream;
  - slot-0 operands stream as k-chunk pieces on BOTH HWDGE rings (sync
    carries b0, scalar carries b2 + b3), slot 0 runs k-major so the
    first matmul needs only chunks 0-1;
  - the diagonal slot (needs no new data) is split: half runs SECOND,
    buying ~1.7us of streaming slack for b3, half runs LAST so the
    final dependency chain is matmul -> ACT -> READ_ACC -> tiny DMA.
"""

import numpy as np
import ml_dtypes

import concourse.mybir as mybir
import concourse.tile as tile
from concourse import bacc
from concourse.bass import ds
from concourse.bass_utils import run_bass_kernel_spmd

N, D_EMB = 4096, 1024
P_PAIRS = N // 2
NCORES = 8
B = 512                    # block size (vertex rows)
KC = D_EMB // 128          # 8 k-chunks
MT = B // 128              # 4 m-tiles per slot
NSLOTS = 8
FRAC_SLOT = 6
DIAG_SLOT = 7
F_FRAC = 384               # fractional slot free dim
MARGIN = 0.5
NBUF = 8
# PE clock warm-up: f32 dummy matmuls issued while slot-0 operands stream
WARM_NS = [128, 128, 64, 64, 64, 64, 64, 64, 64, 64]

# vertices 0-3 = U0-3, 4-7 = V0-3, 8-11 = S0-3
# slot -> (lhs buf index, rhs buf index); slot 6 rhs is the fan buffer FR
SLOT_PATTERN = [(0, 2), (0, 3), (1, 2), (1, 3), (0, 4), (5, 6), (7, None), (2, 2)]
# per-core vertex content of buffers b0..b7 (aliases allowed)
BUFV = [
    [0, 1, 4, 5, 0, 0, 3, 4],
    [2, 3, 6, 7, 2, 1, 3, 4],
    [1, 0, 7, 6, 1, 5, 6, 4],
    [3, 2, 5, 4, 3, 5, 7, 4],
    [0, 1, 8, 9, 1, 6, 7, 8],
    [2, 3, 10, 11, 3, 9, 10, 8],
    [0, 1, 11, 10, 2, 9, 11, 8],
    [2, 3, 9, 8, 1, 10, 11, 8],
]
# fans: cores 0-3 compute rows V0 x cols V1|V2|V3 slice [384c:384c+384];
# cores 4-7 rows S0 x cols S1|S2|S3 slice [384(c-4):...].
V_FAN = (5, 6, 7)
S_FAN = (9, 10, 11)

# execution phases: (slot, tiles).  The diag slot (only needs b2, which
# streams first) is split: tiles 0-1 interleave INTO slot 0's k-groups as
# filler while operand pieces stream; tiles 2-3 run LAST so the final
# dependency chain is matmul -> ACT -> READ_ACC -> tiny DMA with no
# col-sum adds or big output transfers behind it.
PHASES = (
    [(0, (0, 1, 2, 3))]
    + [(s, (0, 1, 2, 3)) for s in range(1, NSLOTS - 1)]
    + [(DIAG_SLOT, (2, 3))]
)


def _slot_edges(c):
    """[(row_vertex, col_vertex_or_None_for_frac)] for core c's 8 slots."""
    out = []
    for s, (l, r) in enumerate(SLOT_PATTERN):
        if s == FRAC_SLOT:
            out.append((BUFV[c][l], None))
        else:
            out.append((BUFV[c][l], BUFV[c][r]))
    return out


def _check_deal():
    need = set()
    for g in ((0, 1, 2, 3), (4, 5, 6, 7), (8, 9, 10, 11)):
        for i, a in enumerate(g):
            for b in g[i:]:
                need.add((a, b))
    for a in (0, 1, 2, 3):
        for b in range(4, 12):
            need.add((a, b))
    got = []
    for c in range(NCORES):
        for s, (a, b) in enumerate(_slot_edges(c)):
            if s == FRAC_SLOT:
                continue
            got.append((min(a, b), max(a, b)))
    fan_edges = [(5, 4), (6, 4), (7, 4), (9, 8), (10, 8), (11, 8)]
    got += [(min(a, b), max(a, b)) for a, b in fan_edges]
    assert sorted(got) == sorted(need), "deal does not cover the 62 blocks"


_check_deal()
_CACHE = {}


def _build_nc():
    nc = bacc.Bacc(
        "TRN2",
        target_bir_lowering=False,
        debug=False,
        num_devices=NCORES,
        enable_partition_id=False,
        monotonic_sem_count=0,
    )
    f32 = mybir.dt.float32
    bf16 = mybir.dt.bfloat16
    fp8 = mybir.dt.float8e4
    bufs_d = nc.dram_tensor("bufs", [NBUF, 128, KC, B], fp8, kind="ExternalInput").ap()
    fan_d = nc.dram_tensor("fan", [128, KC, F_FRAC], fp8, kind="ExternalInput").ap()
    # row sums: col s*4+t = m-tile t of slot s
    out_main = nc.dram_tensor(
        "out_main", [128, NSLOTS * MT], f32, kind="ExternalOutput"
    ).ap()
    # per-slot exp-tile sums (slots 0-6); host reduces partitions
    out_acc = nc.dram_tensor(
        "out_acc", [NSLOTS - 1, 128, B], bf16, kind="ExternalOutput"
    ).ap()
    # second partial col sum of the frac slot, shipped unmerged
    out_acc2 = nc.dram_tensor("out_acc2", [128, B], bf16, kind="ExternalOutput").ap()
    # diag-slot triangle support: exp values above the tri tiles
    out_accd = nc.dram_tensor("out_accd", [128, 256], bf16, kind="ExternalOutput").ap()
    out_acct2 = nc.dram_tensor(
        "out_acct2", [128, 128], bf16, kind="ExternalOutput"
    ).ap()


    with tile.TileContext(nc) as tc:
        with (
            tc.tile_pool(name="xb", bufs=1) as xb_pool,
            tc.tile_pool(name="consts", bufs=1) as consts,
            tc.tile_pool(name="psum", bufs=2, space="PSUM") as psum_pool,
            tc.tile_pool(name="esc", bufs=3) as esc_pool,
            tc.tile_pool(name="stats", bufs=3) as stats,
        ):
            # constants first so the vector queue serves them at body start
            # (the PE warm-up dummies depend on them)
            bias_sb = consts.tile([128, 1], f32, tag="bias")
            nc.vector.memset(bias_sb, MARGIN)
            warm_g = consts.tile([128, 128], f32, tag="warm_g")
            nc.vector.memset(warm_g, 0.0)
            rp_sb = consts.tile([128, NSLOTS * MT], f32, tag="rp")

            # operand buffers
            xbs = [
                xb_pool.tile([128, KC, B], fp8, name=f"xb{b}", tag=f"xb{b}")
                for b in range(NBUF)
            ]
            fan_sb = xb_pool.tile([128, KC, F_FRAC], fp8, tag="fan")

            # slot-0 operands stream as k-chunk pieces on BOTH HWDGE rings
            # (sync carries b0, scalar carries b2 then b3); the other
            # buffers stream whole, ordered by slot consumption time.
            # b5/b7 issue from the scalar ring later (between ACT groups).
            for h in range(KC // 2):
                nc.sync.dma_start(
                    out=xbs[0][:, 2 * h : 2 * h + 2, :],
                    in_=bufs_d[0, :, 2 * h : 2 * h + 2, :],
                )
                nc.scalar.dma_start(
                    out=xbs[2][:, 2 * h : 2 * h + 2, :],
                    in_=bufs_d[2, :, 2 * h : 2 * h + 2, :],
                )
            # b3 gates slot 1 right after the merged phase — split it
            # across both rings so it lands in half the time
            nc.sync.dma_start(out=xbs[3][:, 0:4, :], in_=bufs_d[3, :, 0:4, :])
            nc.scalar.dma_start(out=xbs[3][:, 4:8, :], in_=bufs_d[3, :, 4:8, :])
            for b in (1, 4, 6):
                nc.sync.dma_start(out=xbs[b], in_=bufs_d[b])
            nc.sync.dma_start(out=fan_sb, in_=fan_d)

            # warm the exp table set (forces the ACT_TABLE_LOAD early)
            warm = consts.tile([128, 1], f32, tag="warm")
            nc.scalar.activation(warm, bias_sb, mybir.ActivationFunctionType.Exp)

            # PE clock warm-up: f32 dummy matmuls on const data keep the PE
            # continuously busy from body start so the p-state ramp (full
            # clock after ~3us of activity) completes before real operands
            # arrive.  They write a scratch generation of the dps0 psum tag;
            # real slot matmuls start=True overwrite it.
            warm_ps = psum_pool.tile([128, B], f32, name="warm_ps", tag="dps0")
            for wn in WARM_NS:
                nc.tensor.matmul(
                    warm_ps[0:1, 0:wn],
                    bias_sb[:, 0:1],
                    warm_g[:, 0:wn],
                )

            def emit_act(slot, esc, lane, t, dpsum, F):
                """exp ACT for m-tile t; tiles >=2 row-sum via accumulator."""
                col = slot * MT + t
                if t >= 2:
                    nc.scalar.activation(
                        esc[:, lane, :F],
                        dpsum[:, :F],
                        mybir.ActivationFunctionType.Exp,
                        bias=bias_sb,
                        scale=1.0,
                        accum_out=rp_sb[:, col : col + 1],
                    )
                else:
                    nc.scalar.activation(
                        esc[:, lane, :F],
                        dpsum[:, :F],
                        mybir.ActivationFunctionType.Exp,
                        bias=bias_sb,
                        scale=1.0,
                    )

            # --- phase 0: slot 0 k-major, with diag tiles 0-1 (only need
            # b2, the first-streamed buffer) interleaved into the k-group
            # gaps so the PE never idles while pieces stream in.
            esc0 = esc_pool.tile([128, MT, B], bf16, tag="esc")
            escd = esc_pool.tile([128, MT, B], bf16, tag="esc")
            dps0 = {
                t: psum_pool.tile([128, B], f32, name=f"dps{t}", tag=f"dps{t}")
                for t in range(MT)
            }
            dpsd = {
                t: psum_pool.tile([128, B], f32, name=f"dpsd{t}", tag=f"dps{t}")
                for t in (0, 1)
            }
            xl0 = xbs[SLOT_PATTERN[0][0]]
            xr0 = xbs[SLOT_PATTERN[0][1]]
            xd = xbs[SLOT_PATTERN[DIAG_SLOT][0]]
            for k2 in range(KC // 2):
                last = k2 == KC // 2 - 1
                for t in range(MT):
                    nc.tensor.matmul(
                        dps0[t],
                        xl0[:, 2 * k2 : 2 * k2 + 2, ds(128 * t, 128)],
                        xr0[:, 2 * k2 : 2 * k2 + 2, :],
                        start=(k2 == 0),
                        stop=last,
                        perf_mode=mybir.MatmulPerfMode.DoubleRow,
                    )
                    if last:
                        emit_act(0, esc0, t, t, dps0[t], B)
                for t in (0, 1):
                    nc.tensor.matmul(
                        dpsd[t],
                        xd[:, 2 * k2 : 2 * k2 + 2, ds(128 * t, 128)],
                        xd[:, 2 * k2 : 2 * k2 + 2, :],
                        start=(k2 == 0),
                        stop=last,
                        perf_mode=mybir.MatmulPerfMode.DoubleRow,
                    )
                    if last:
                        emit_act(DIAG_SLOT, escd, t, t, dpsd[t], B)
            nc.vector.tensor_reduce(
                rp_sb[:, 0:2],
                esc0[:, 0:2, :],
                axis=mybir.AxisListType.X,
                op=mybir.AluOpType.add,
            )
            nc.vector.tensor_reduce(
                rp_sb[:, DIAG_SLOT * MT : DIAG_SLOT * MT + 2],
                escd[:, 0:2, :],
                axis=mybir.AxisListType.X,
                op=mybir.AluOpType.add,
            )
            # right half of the diag rows 0:256, summed — feeds the host's
            # below-diagonal column sums for the triangular tiles 2-3
            accd = stats.tile([128, 256], bf16, tag="accd")
            nc.vector.tensor_add(
                accd, escd[:, 0, 256:512], escd[:, 1, 256:512]
            )
            nc.sync.dma_start(out=out_accd, in_=accd)
            acc01 = stats.tile([128, B], bf16, tag="acc01")
            nc.vector.tensor_add(acc01, esc0[:, 0, :], esc0[:, 1, :])
            acc23 = stats.tile([128, B], bf16, tag="acc23")
            nc.gpsimd.tensor_add(acc23, esc0[:, 2, :], esc0[:, 3, :])
            accf = stats.tile([128, B], bf16, tag="accf")
            nc.vector.tensor_add(accf, acc01, acc23)
            nc.sync.dma_start(out=out_acc[0], in_=accf)

            # --- remaining phases: diag tiles 2-3 (covers the b3 stream),
            # then slots 1..6 m-tile-major; frac slot last.
            for p, (s, tiles) in enumerate(PHASES[1:]):
                li, ri = SLOT_PATTERN[s]
                xl = xbs[li]
                xr = fan_sb if s == FRAC_SLOT else xbs[ri]
                F = F_FRAC if s == FRAC_SLOT else B
                # late operand issues ride the scalar ring between ACT groups
                if p == 2:
                    nc.scalar.dma_start(out=xbs[5], in_=bufs_d[5])
                if p == 3:
                    nc.scalar.dma_start(out=xbs[7], in_=bufs_d[7])
                esc = esc_pool.tile([128, MT, B], bf16, tag="esc")
                dpsums = {}
                for t in tiles:
                    dpsums[t] = psum_pool.tile(
                        [128, B], f32, name=f"dps{t}", tag=f"dps{t}"
                    )
                for t in tiles:
                    if s == DIAG_SLOT:
                        # triangular tiles: only cols >= the tile's row
                        # block; the below-diagonal part comes from the
                        # shipped exp tiles by symmetry (host side)
                        c0, w = 128 * t, B - 128 * t
                    else:
                        c0, w = 0, F
                    for k2 in range(KC // 2):
                        nc.tensor.matmul(
                            dpsums[t][:, :w],
                            xl[:, 2 * k2 : 2 * k2 + 2, ds(128 * t, 128)],
                            xr[:, 2 * k2 : 2 * k2 + 2, ds(c0, w)],
                            start=(k2 == 0),
                            stop=(k2 == KC // 2 - 1),
                            perf_mode=mybir.MatmulPerfMode.DoubleRow,
                        )
                    emit_act(s, esc, t - tiles[0], t, dpsums[t], w)
                    if s == DIAG_SLOT and t == 2:
                        # rows 256:384 x cols 384:512 -> host col sums for
                        # the last tile's rows
                        nc.sync.dma_start(
                            out=out_acct2, in_=esc[:, 0, 128:256]
                        )
                # row sums for m-tiles 0-1 via one DVE free-dim reduce
                if tiles[0] == 0:
                    nc.vector.tensor_reduce(
                        rp_sb[:, s * MT : s * MT + 2],
                        esc[:, 0:2, :F],
                        axis=mybir.AxisListType.X,
                        op=mybir.AluOpType.add,
                    )
                # col sums (not needed for the diag slot)
                if s == DIAG_SLOT:
                    continue
                acc01 = stats.tile([128, B], bf16, tag="acc01")
                nc.vector.tensor_add(acc01[:, :F], esc[:, 0, :F], esc[:, 1, :F])
                acc23 = stats.tile([128, B], bf16, tag="acc23")
                if s == FRAC_SLOT:
                    # 2nd-to-last slot: its chain gates the final barrier.
                    # Ship the two partial sums separately (acc01 as soon
                    # as tiles 0-1 are done, acc23 off the slow gpsimd,
                    # no third add) so only acc23's small DMA is late.
                    nc.sync.dma_start(out=out_acc[s][:, :F], in_=acc01[:, :F])
                    nc.vector.tensor_add(acc23[:, :F], esc[:, 2, :F], esc[:, 3, :F])
                    nc.sync.dma_start(out=out_acc2[:, :F], in_=acc23[:, :F])
                else:
                    nc.gpsimd.tensor_add(acc23[:, :F], esc[:, 2, :F], esc[:, 3, :F])
                    accf = stats.tile([128, B], bf16, tag="accf")
                    nc.vector.tensor_add(accf[:, :F], acc01[:, :F], acc23[:, :F])
                    nc.sync.dma_start(out=out_acc[s][:, :F], in_=accf[:, :F])
            # row sums out: single DMA once the last READ_ACC lands; issued
            # from the scalar ring, which is idle right after that READ
            nc.scalar.dma_start(out=out_main, in_=rp_sb)
    nc.compile()
    return nc


def _get_nc():
    if "nc" not in _CACHE:
        _CACHE["nc"] = _build_nc()
    return _CACHE["nc"]


def _prep_host(text_embeddings, shape_embeddings):
    """-> (W fp64 [12*512, 1024], per-core in_maps)."""
    T = np.asarray(text_embeddings, dtype=np.float32)
    S = np.asarray(shape_embeddings, dtype=np.float32)
    W = np.concatenate([T[0::2], T[1::2], S], axis=0)  # [6144, 1024]
    # [128, KC, 6144] fp8 view of W^T: partition p, chunk kc = d-row kc*128+p
    WT = np.ascontiguousarray(W.T).astype(ml_dtypes.float8_e4m3)
    Wg = WT.reshape(KC, 128, 12 * B).transpose(1, 0, 2)
    Wg = np.ascontiguousarray(Wg)
    in_maps = []
    for c in range(NCORES):
        bufs = np.empty((NBUF, 128, KC, B), dtype=ml_dtypes.float8_e4m3)
        for b in range(NBUF):
            v = BUFV[c][b]
            bufs[b] = Wg[:, :, B * v : B * (v + 1)]
        fan_vs = V_FAN if c < 4 else S_FAN
        fan_cols = np.concatenate(
            [Wg[:, :, B * v : B * (v + 1)] for v in fan_vs], axis=2
        )
        k = (c % 4) * F_FRAC
        fan = np.ascontiguousarray(fan_cols[:, :, k : k + F_FRAC])
        in_maps.append({"bufs": bufs, "fan": fan})
    return W.astype(np.float64), in_maps


def _finalize(W, outs):
    """W: [6144,1024] fp64; outs: 8 per-core output dicts -> scalar loss."""
    # per-vertex row-sum accumulators for each loss
    rs = [np.zeros((12, B), np.float64), np.zeros((12, B), np.float64)]

    def classes(a, b):
        ga, gb = a // 4, b // 4
        if ga == 0 and gb == 0:
            return (0, 1)
        return (0,) if (ga <= 1 and gb <= 1) else (1,)

    for c, o in enumerate(outs):
        rp = np.asarray(o["out_main"], np.float64)          # [128, 32]
        acc = np.asarray(o["out_acc"], np.float32).astype(np.float64)
        cs = acc.sum(axis=1)                                 # [7, 512] col sums
        # frac slot shipped acc01 (in out_acc[6]) and acc23 separately
        cs[FRAC_SLOT] += (
            np.asarray(o["out_acc2"], np.float32).astype(np.float64).sum(axis=0)
        )
        edges = _slot_edges(c)
        # diag-slot triangle: below-diagonal col sums from shipped exp tiles
        accd_s = np.asarray(o["out_accd"], np.float32).astype(np.float64).sum(axis=0)
        acct2_s = np.asarray(o["out_acct2"], np.float32).astype(np.float64).sum(axis=0)
        for s in range(NSLOTS):
            a, b = edges[s]
            F = F_FRAC if s == FRAC_SLOT else B
            # row sums -> vertex a
            for t in range(MT):
                seg = rp[:, s * MT + t]
                if s == DIAG_SLOT and t == 2:
                    seg = seg + accd_s[0:128]
                elif s == DIAG_SLOT and t == 3:
                    seg = seg + accd_s[128:256] + acct2_s
                if s == FRAC_SLOT:
                    for l in classes(a, a):
                        rs[l][a][128 * t : 128 * (t + 1)] += seg
                else:
                    for l in classes(a, b):
                        rs[l][a][128 * t : 128 * (t + 1)] += seg
            # col sums -> vertex b (skip diag blocks and the diag slot)
            if s == DIAG_SLOT:
                continue
            if s == FRAC_SLOT:
                fan_vs = V_FAN if c < 4 else S_FAN
                k = (c % 4) * F_FRAC
                for j0 in range(F):
                    g = k + j0
                    v = fan_vs[g // B]
                    for l in classes(a, v):
                        rs[l][v][g % B] += cs[s, j0]
            elif a != b:
                for l in classes(a, b):
                    rs[l][b] += cs[s, :F]

    U, V, S = W[0:2048], W[2048:4096], W[4096:6144]
    total = 0.0
    for l, (A_, B_) in enumerate(((U, V), (U, S))):
        dab = np.einsum("ij,ij->i", A_, B_)
        daa = np.einsum("ij,ij->i", A_, A_)
        dbb = np.einsum("ij,ij->i", B_, B_)
        rsB = rs[l][4:8].reshape(-1) if l == 0 else rs[l][8:12].reshape(-1)
        Sp = (
            rs[l][0:4].reshape(-1)
            + rsB
            - 2.0 * np.exp(MARGIN + dab)
            - np.exp(MARGIN + daa)
            - np.exp(MARGIN + dbb)
        )
        J = np.square(np.maximum(np.log(Sp) - dab, 0.0))
        loss = J.sum() / P_PAIRS / 2.0
        total += loss if l == 0 else 2.0 * loss
    return np.asarray(total, dtype=np.float32)


def kernel(text_embeddings, shape_embeddings):
    W, in_maps = _prep_host(text_embeddings, shape_embeddings)
    nc = _get_nc()
    res = run_bass_kernel_spmd(nc, in_maps, core_ids=list(range(NCORES)))
    return _finalize(W, res.results)
